# revision 11
# baseline (speedup 1.0000x reference)
"""CfC Liquid Cell kernel for Trainium2 (Bass/Tile), 8 NeuronCores.

Sharding: data-parallel over batch (B=8 -> 1 batch element per core).

Fully fused single-pass design (no DRAM scratch):
  loop over NA=4 chunks of TA=512 timesteps:
    - x arrives feature-major via XBAR DMA-transpose (bf16), no PE transpose
    - in_proj (bf16 weights stationary, xT moving) -> xp (conv input, bf16)
      and zs = silu(z) (bf16)
    - depthwise causal conv = 4 shifted diagonal matmuls, conv bias folded
      into the silu activation bias
    - per 256-wide sub-chunk (software-pipelined one sub-chunk behind):
      gates bb/f1/f2/T/D via 2-head block-diagonal matmuls + ACT tanh/silu
      (sigmoid via tanh: ACT scale=0.5), fused DVE/Pool elementwise:
        C  = (1+T)*f2 - (T-1)*f1          (= 2*candidate)
        ch = (D-1)*C                      (= -4*(1-d)*candidate)
        d  = 0.5*D + 0.5
        g  = scan(g = d*g + ch)           (= -4*h; -1/4 folded into W_so)
      s3 (lag 1 sub): state_out matmul; bias+z-gating fused into one
      scalar_tensor_tensor; out_proj (lag 2 subs) DMAs straight from PSUM.

All matmul operands bf16 (1 cycle/row, same as f32r at >=256 moving rows,
but half the LDWEIGHTS/DMA/SBUF cost); PSUM accumulation stays fp32; the
scan's internal state is fp32 by hardware.
"""

import numpy as np

B, S, H = 8, 2048, 1024
NH, HD, NS, K = 16, 64, 64, 4
N_CORES = 8
P = 128
TA = 512            # phase-1 chunk (transpose/in_proj/conv)
TB = 256            # phase-2 sub-chunk (gates/scan/out_proj)
NA = S // TA        # 4
NB = S // TB        # 8

_CACHE = {}


def _build_program():
    import concourse.bacc as bacc
    import concourse.mybir as mybir
    import concourse.tile as tile

    F32 = mybir.dt.float32
    BF16 = mybir.dt.bfloat16
    AF = mybir.ActivationFunctionType
    ALU = mybir.AluOpType

    nc = bacc.Bacc("TRN2", target_bir_lowering=False, debug=False)

    x_d = nc.dram_tensor("x", (S, H), BF16, kind="ExternalInput").ap()
    w_in_d = nc.dram_tensor("w_in", (P, 8, 2 * H), BF16, kind="ExternalInput").ap()
    cdiag_d = nc.dram_tensor("cdiag", (P, 8, K, P), BF16, kind="ExternalInput").ap()
    cbias_d = nc.dram_tensor("cbias", (P, 8), F32, kind="ExternalInput").ap()
    blk_d = nc.dram_tensor("blk", (P, 6, P), BF16, kind="ExternalInput").ap()
    bias_d = nc.dram_tensor("bias", (P, 6), F32, kind="ExternalInput").ap()
    w_out_d = nc.dram_tensor("w_out", (P, 8, H), BF16, kind="ExternalInput").ap()
    y_d = nc.dram_tensor("y", (S, H), F32, kind="ExternalOutput").ap()

    with tile.TileContext(nc) as tc:
        with tc.tile_pool(name="cw", bufs=1) as cw, \
             tc.tile_pool(name="sb", bufs=1) as sb, \
             tc.tile_pool(name="wk", bufs=20) as wk, \
             tc.tile_pool(name="ps", bufs=3, space="PSUM") as ps, \
             tc.tile_pool(name="psy", bufs=2, space="PSUM") as psy:

            w_in = cw.tile([P, 8, 2 * H], BF16)
            nc.scalar.dma_start(w_in[:], w_in_d[:])
            cdiag = cw.tile([P, 8, K, P], BF16)
            nc.scalar.dma_start(cdiag[:], cdiag_d[:])
            blk = cw.tile([P, 6, P], BF16)
            nc.scalar.dma_start(blk[:], blk_d[:])
            cbias = cw.tile([P, 8], F32)
            nc.scalar.dma_start(cbias[:], cbias_d[:])
            bias = cw.tile([P, 6], F32)
            nc.scalar.dma_start(bias[:], bias_d[:])
            w_out = cw.tile([P, 8, H], BF16)
            nc.scalar.dma_start(w_out[:], w_out_d[:])

            def dma_xT(i):
                xT = sb.tile([P, 8, TA], BF16, tag="xT", bufs=2, name="xT")
                for kt in range(8):
                    nc.sync.dma_start(
                        xT[:, kt, :],
                        x_d[i * TA:(i + 1) * TA, kt * P:(kt + 1) * P],
                        transpose=True)
                return xT

            def phase1(i, xT, xp_prev):
                xp = sb.tile([P, 8, K - 1 + TA], BF16, tag="xp", bufs=2,
                             name="xp")
                if i == 0:
                    nc.vector.memset(xp[:, :, :K - 1], 0.0)
                else:
                    nc.vector.tensor_copy(xp[:, :, :K - 1],
                                          xp_prev[:, :, TA:TA + K - 1])
                zs = sb.tile([P, 8, TA], BF16, tag="zs", bufs=2, name="zs")
                xh = sb.tile([P, 8, TA], BF16, tag="xh", bufs=2, name="xh")
                # x-path half first so conv can start as soon as possible
                for jt in range(8):
                    pm = ps.tile([P, TA], F32, tag="mm", name="pm")
                    for kt in range(8):
                        nc.tensor.matmul(
                            pm[:], w_in[:, kt, jt * P:(jt + 1) * P],
                            xT[:, kt, :], start=(kt == 0), stop=(kt == 7))
                    nc.scalar.activation(xp[:, jt, K - 1:], pm[:], AF.Copy)
                for jt in range(8, 16):
                    pm = ps.tile([P, TA], F32, tag="mm", name="pm")
                    for kt in range(8):
                        nc.tensor.matmul(
                            pm[:], w_in[:, kt, jt * P:(jt + 1) * P],
                            xT[:, kt, :], start=(kt == 0), stop=(kt == 7))
                    nc.scalar.activation(zs[:, jt - 8, :], pm[:], AF.Silu)
                for ct in range(8):
                    pc = ps.tile([P, TA], F32, tag="mm", name="pc")
                    for tap in range(K):
                        nc.tensor.matmul(
                            pc[:], cdiag[:, ct, tap, :],
                            xp[:, ct, tap:tap + TA],
                            start=(tap == 0), stop=(tap == K - 1))
                    nc.scalar.activation(xh[:, ct, :], pc[:], AF.Silu,
                                         bias=cbias[:, ct:ct + 1])
                return xp, zs, xh

            def stage(widx, rhs, lane0, off, out, func, scale=1.0):
                for q in range(2):
                    pg = ps.tile([P, 2, TB], F32, tag="mm", name="pg")
                    nc.tensor.matmul(
                        pg[:], blk[:, widx, :],
                        rhs[:, lane0 + 2 * q:lane0 + 2 * q + 2, off:off + TB],
                        start=True, stop=True)
                    nc.scalar.activation(out[:, 2 * q:2 * q + 2, :], pg[:],
                                         func, bias=bias[:, widx:widx + 1],
                                         scale=scale)

            def gates_scan(j, xh, off, h, h_prev, off_prev):
                for hf in range(2):
                    p0 = 4 * hf
                    bbh = wk.tile([P, 4, TB], BF16, tag="wk", name="bbh")
                    stage(0, xh, p0, off, bbh, AF.Silu)
                    f1 = wk.tile([P, 4, TB], BF16, tag="wk", name="f1")
                    stage(1, bbh, 0, 0, f1, AF.Tanh)
                    f2 = wk.tile([P, 4, TB], BF16, tag="wk", name="f2")
                    stage(2, bbh, 0, 0, f2, AF.Tanh)
                    tt = wk.tile([P, 4, TB], BF16, tag="wk", name="tt")
                    stage(3, bbh, 0, 0, tt, AF.Tanh, scale=0.5)
                    td = wk.tile([P, 4, TB], BF16, tag="wk", name="td")
                    stage(4, bbh, 0, 0, td, AF.Tanh, scale=0.5)

                    aa = wk.tile([P, 4, TB], BF16, tag="wk", name="aa")
                    nc.vector.scalar_tensor_tensor(
                        aa[:], tt[:], 1.0, f2[:], ALU.add, ALU.mult)
                    bb2 = wk.tile([P, 4, TB], BF16, tag="wk", name="bb2")
                    nc.vector.scalar_tensor_tensor(
                        bb2[:], tt[:], -1.0, f1[:], ALU.add, ALU.mult)
                    cc = wk.tile([P, 4, TB], BF16, tag="wk", name="cc")
                    nc.gpsimd.tensor_tensor(cc[:], aa[:], bb2[:], ALU.subtract)
                    ch = wk.tile([P, 4, TB], BF16, tag="wk", name="ch")
                    nc.vector.scalar_tensor_tensor(
                        ch[:], td[:], -1.0, cc[:], ALU.add, ALU.mult)
                    dd = wk.tile([P, 4, TB], BF16, tag="wk", name="dd")
                    nc.vector.tensor_scalar(dd[:], td[:], 0.5, 0.5,
                                            ALU.mult, ALU.add)
                    for lq in range(4):
                        lt = p0 + lq
                        init = (0.0 if j == 0 else
                                h_prev[:, lt, off_prev + TB - 1:off_prev + TB])
                        nc.vector.tensor_tensor_scan(
                            h[:, lt, off:off + TB], dd[:, lq, :],
                            ch[:, lq, :], init, ALU.mult, ALU.add)

            def s3_gate(j, h, zs, off):
                ghs = []
                for hf in range(2):
                    p0 = 4 * hf
                    gh = wk.tile([P, 4, TB], BF16, tag="gh", bufs=6, name="gh")
                    for q in range(2):
                        pg = ps.tile([P, 2, TB], F32, tag="mm", name="pg")
                        nc.tensor.matmul(
                            pg[:], blk[:, 5, :],
                            h[:, p0 + 2 * q:p0 + 2 * q + 2, off:off + TB],
                            start=True, stop=True)
                        nc.vector.scalar_tensor_tensor(
                            gh[:, 2 * q:2 * q + 2, :], pg[:], bias[:, 5:6],
                            zs[:, p0 + 2 * q:p0 + 2 * q + 2, off:off + TB],
                            ALU.add, ALU.mult)
                    ghs.append(gh)
                return ghs

            def outproj(j, ghs):
                for st in range(TB // P):
                    py = psy.tile([P, H], F32, tag="py", name="py")
                    for kt in range(8):
                        lh = ghs[kt // 4][:, kt % 4, st * P:(st + 1) * P]
                        nc.tensor.matmul(py[:, 0:512], lh,
                                         w_out[:, kt, 0:512],
                                         start=(kt == 0), stop=(kt == 7))
                        nc.tensor.matmul(py[:, 512:H], lh,
                                         w_out[:, kt, 512:H],
                                         start=(kt == 0), stop=(kt == 7))
                    ysb = sb.tile([P, H], F32, tag="ysb", bufs=4, name="ysb")
                    nc.vector.tensor_copy(ysb[:], py[:])
                    nc.scalar.dma_start(
                        y_d[j * TB + st * P:j * TB + (st + 1) * P, :], ysb[:])

            # ---- software-pipelined main loop ----
            xT_next = dma_xT(0)
            xp_prev = None
            h_prev_info = (None, 0)
            pend_s3 = {}   # j -> (h, zs, off)
            pend_op = {}   # j -> ghs
            for i in range(NA):
                xT = xT_next
                if i + 1 < NA:
                    xT_next = dma_xT(i + 1)
                xp_prev, zs, xh = phase1(i, xT, xp_prev)
                h = sb.tile([P, 8, TA], BF16, tag="h", bufs=2, name="h")
                for s_ in range(2):
                    j = 2 * i + s_
                    off = s_ * TB
                    hp, offp = h_prev_info
                    gates_scan(j, xh, off, h, hp, offp)
                    h_prev_info = (h, off)
                    pend_s3[j] = (h, zs, off)
                    # out_proj before s3: gives the j-1 scan chain (ACT->DVE)
                    # a full out_proj of slack before PE needs h(j-1)
                    if j - 2 in pend_op:
                        outproj(j - 2, pend_op.pop(j - 2))
                    if j - 1 in pend_s3:
                        jj = j - 1
                        hh, zz, oo = pend_s3.pop(jj)
                        pend_op[jj] = s3_gate(jj, hh, zz, oo)
            # drain
            jj = NB - 1
            hh, zz, oo = pend_s3.pop(jj)
            pend_op[jj] = s3_gate(jj, hh, zz, oo)
            outproj(NB - 2, pend_op.pop(NB - 2))
            outproj(NB - 1, pend_op.pop(NB - 1))

    nc.compile()
    return nc


def _to_bf16(a):
    import ml_dtypes
    return np.asarray(a, np.float32).astype(ml_dtypes.bfloat16)


def _prep_shared(inputs):
    """Host-side preprocessing of the shared (weight) tensors."""
    f32 = np.float32
    in_proj_w = np.asarray(inputs["in_proj_w"], f32)
    conv_w = np.asarray(inputs["conv_w"], f32)
    conv_b = np.asarray(inputs["conv_b"], f32)

    w_in = in_proj_w.reshape(8, P, 2 * H).transpose(1, 0, 2)
    w_out = np.asarray(inputs["out_proj_w"], f32).reshape(8, P, H).transpose(1, 0, 2)

    cdiag = np.zeros((8, K, P, P), f32)
    rng = np.arange(P)
    for ct in range(8):
        for tap in range(K):
            cdiag[ct, tap, rng, rng] = conv_w[ct * P:(ct + 1) * P, 0, tap]
    cdiag = cdiag.transpose(2, 0, 1, 3)  # (P, 8, K, P)
    cbias = conv_b.reshape(8, P).T  # (P, 8)

    def blk2(w):
        o = np.zeros((P, P), f32)
        o[:64, :64] = w
        o[64:, 64:] = w
        return o

    blk = np.stack([
        blk2(np.asarray(inputs["bb_w"], f32)),
        blk2(np.asarray(inputs["f1_w"], f32)),
        blk2(np.asarray(inputs["f2_w"], f32)),
        blk2(np.asarray(inputs["tau_a_w"], f32)),
        blk2(np.asarray(inputs["decay_w"], f32)),
        blk2(np.asarray(inputs["state_out_w"], f32) * (-0.25)),
    ], axis=1)  # (P, 6, P)

    def t2(v):
        return np.tile(np.asarray(v, f32), 2)

    bias = np.stack([
        t2(inputs["bb_b"]),
        t2(inputs["f1_b"]),
        t2(inputs["f2_b"]),
        0.5 * (t2(inputs["tau_a_b"]) + t2(inputs["tau_b"])),
        0.5 * t2(inputs["decay_b"]),
        t2(inputs["state_out_b"]),
    ], axis=1)  # (P, 6)

    return {
        "w_in": np.ascontiguousarray(_to_bf16(w_in)),
        "w_out": np.ascontiguousarray(_to_bf16(w_out)),
        "cdiag": np.ascontiguousarray(_to_bf16(cdiag)),
        "cbias": np.ascontiguousarray(cbias),
        "blk": np.ascontiguousarray(_to_bf16(blk)),
        "bias": np.ascontiguousarray(bias),
    }


def _make_in_maps(inputs):
    shared = _prep_shared(inputs)
    x = _to_bf16(inputs["x"])
    in_maps = []
    for b in range(N_CORES):
        m = dict(shared)
        m["x"] = np.ascontiguousarray(x[b])
        in_maps.append(m)
    return in_maps


def kernel(**inputs) -> np.ndarray:
    from concourse import bass_utils

    if "nc" not in _CACHE:
        _CACHE["nc"] = _build_program()
    nc = _CACHE["nc"]

    in_maps = _make_in_maps(inputs)
    res = bass_utils.run_bass_kernel_spmd(nc, in_maps, core_ids=list(range(N_CORES)))
    out = np.stack([res.results[b]["y"] for b in range(N_CORES)], axis=0)
    return out.astype(np.float32)


# tau/decay note: sigmoid(u) = 0.5 + 0.5*tanh(u/2). ACT computes
# tanh(0.5*psum + 0.5*b) via scale=0.5 and a pre-halved bias column; the
# remaining affine is folded into the fused DVE ops:
#   candidate*2 = C = (1+T)*f2 - (T-1)*f1
#   scan input ch = (D-1)*C = -4*(1-d)*candidate, so the scan state is
#   g = -4*h and -1/4 is folded into the state_out weights host-side.


# revision 12
# speedup vs baseline: 1.0605x; 1.0605x over previous
"""CfC Liquid Cell kernel for Trainium2 (Bass/Tile), 8 NeuronCores.

Sharding: data-parallel over batch (B=8 -> 1 batch element per core).

Fully fused single-pass design (no DRAM scratch):
  loop over NA=4 chunks of TA=512 timesteps:
    - x arrives feature-major via XBAR DMA-transpose (bf16), no PE transpose
    - in_proj (bf16 weights stationary, xT moving) -> xp (conv input, bf16)
      and zs = silu(z) (bf16)
    - depthwise causal conv = 4 shifted diagonal matmuls, conv bias folded
      into the silu activation bias
    - per 256-wide sub-chunk, software-pipelined one full chunk behind
      phase 1 (phase1(i+1) is issued before chunk i's sub-chunks so the
      in_proj matmuls overlap the gate/scan chains):
      gates bb/f1/f2/T/D via 2-head block-diagonal matmuls + ACT tanh/silu
      (sigmoid via tanh: ACT scale=0.5), elementwise on DVE/Pool using only
      ops with fast 2x/4x bf16 modes (tensor_tensor / tensor_scalar):
        v=f2-f1; s=f1+f2; w=T*v; C=s+w            (= 2*candidate)
        dneg=0.5-0.5D (=1-d); dd=0.5+0.5D (=d); ch=dneg*C
        g = scan(g = dd*g + ch)                   (= 2*h; 0.5 folded in W_so)
      s3 (lag 1 sub): state_out matmul -> ACT Identity+bias -> oseq;
      gh = oseq*zs on Pool; out_proj (lag 2 subs) -> bf16 staging -> DMA.

All matmul operands bf16 (1 cycle/row, same as f32r at >=256 moving rows,
but half the LDWEIGHTS/DMA/SBUF cost); PSUM accumulation stays fp32; the
scan's internal state is fp32 by hardware.
"""

import numpy as np

B, S, H = 8, 2048, 1024
NH, HD, NS, K = 16, 64, 64, 4
N_CORES = 8
P = 128
TA = 512            # phase-1 chunk (transpose/in_proj/conv)
TB = 256            # phase-2 sub-chunk (gates/scan/out_proj)
NA = S // TA        # 4
NB = S // TB        # 8

_CACHE = {}


def _build_program():
    import concourse.bacc as bacc
    import concourse.mybir as mybir
    import concourse.tile as tile

    F32 = mybir.dt.float32
    BF16 = mybir.dt.bfloat16
    AF = mybir.ActivationFunctionType
    ALU = mybir.AluOpType

    nc = bacc.Bacc("TRN2", target_bir_lowering=False, debug=False)

    x_d = nc.dram_tensor("x", (S, H), BF16, kind="ExternalInput").ap()
    w_in_d = nc.dram_tensor("w_in", (P, 8, 2 * H), BF16, kind="ExternalInput").ap()
    cdiag_d = nc.dram_tensor("cdiag", (P, 8, K, P), BF16, kind="ExternalInput").ap()
    cbias_d = nc.dram_tensor("cbias", (P, 8), F32, kind="ExternalInput").ap()
    blk_d = nc.dram_tensor("blk", (P, 6, P), BF16, kind="ExternalInput").ap()
    bias_d = nc.dram_tensor("bias", (P, 6), F32, kind="ExternalInput").ap()
    w_out_d = nc.dram_tensor("w_out", (P, 8, H), BF16, kind="ExternalInput").ap()
    y_d = nc.dram_tensor("y", (S, H), BF16, kind="ExternalOutput").ap()

    with tile.TileContext(nc) as tc:
        with tc.tile_pool(name="cw", bufs=1) as cw, \
             tc.tile_pool(name="sb", bufs=1) as sb, \
             tc.tile_pool(name="wk", bufs=20) as wk, \
             tc.tile_pool(name="ps", bufs=4, space="PSUM") as ps, \
             tc.tile_pool(name="psy", bufs=2, space="PSUM") as psy:

            # prefetch ACT tables with dummy activations (overlaps weight DMA)
            warm = sb.tile([P, 2], BF16, tag="warm", bufs=1, name="warm")
            nc.vector.memset(warm[:], 0.0)
            nc.scalar.activation(warm[:, 0:1], warm[:, 0:1], AF.Silu)
            nc.scalar.activation(warm[:, 1:2], warm[:, 1:2], AF.Tanh)

            w_in = cw.tile([P, 8, 2 * H], BF16)
            # x-path half first: first in_proj matmuls need only jt 0..7
            nc.scalar.dma_start(w_in[:, :, 0:H], w_in_d[:, :, 0:H])
            nc.scalar.dma_start(w_in[:, :, H:2 * H], w_in_d[:, :, H:2 * H])
            cdiag = cw.tile([P, 8, K, P], BF16)
            nc.scalar.dma_start(cdiag[:], cdiag_d[:])
            blk = cw.tile([P, 6, P], BF16)
            nc.scalar.dma_start(blk[:], blk_d[:])
            cbias = cw.tile([P, 8], F32)
            nc.scalar.dma_start(cbias[:], cbias_d[:])
            bias = cw.tile([P, 6], F32)
            nc.scalar.dma_start(bias[:], bias_d[:])
            w_out = cw.tile([P, 8, H], BF16)
            nc.scalar.dma_start(w_out[:], w_out_d[:])

            def dma_xT(i):
                xT = sb.tile([P, 8, TA], BF16, tag="xT", bufs=2, name="xT")
                for kt in range(8):
                    nc.sync.dma_start(
                        xT[:, kt, :],
                        x_d[i * TA:(i + 1) * TA, kt * P:(kt + 1) * P],
                        transpose=True)
                return xT

            def phase1(i, xT, xp_prev):
                xp = sb.tile([P, 8, K - 1 + TA], BF16, tag="xp", bufs=2,
                             name="xp")
                if i == 0:
                    nc.vector.memset(xp[:, :, :K - 1], 0.0)
                else:
                    nc.vector.tensor_copy(xp[:, :, :K - 1],
                                          xp_prev[:, :, TA:TA + K - 1])
                zs = sb.tile([P, 8, TA], BF16, tag="zs", bufs=3, name="zs")
                xh = sb.tile([P, 8, TA], BF16, tag="xh", bufs=2, name="xh")
                # x-path half first so conv can start as soon as possible
                for jt in range(8):
                    pm = ps.tile([P, TA], F32, tag="mm", name="pm")
                    for kt in range(8):
                        nc.tensor.matmul(
                            pm[:], w_in[:, kt, jt * P:(jt + 1) * P],
                            xT[:, kt, :], start=(kt == 0), stop=(kt == 7))
                    nc.scalar.activation(xp[:, jt, K - 1:], pm[:], AF.Copy)
                for jt in range(8, 16):
                    pm = ps.tile([P, TA], F32, tag="mm", name="pm")
                    for kt in range(8):
                        nc.tensor.matmul(
                            pm[:], w_in[:, kt, jt * P:(jt + 1) * P],
                            xT[:, kt, :], start=(kt == 0), stop=(kt == 7))
                    nc.scalar.activation(zs[:, jt - 8, :], pm[:], AF.Silu)
                for ct in range(8):
                    pc = ps.tile([P, TA], F32, tag="mm", name="pc")
                    for tap in range(K):
                        nc.tensor.matmul(
                            pc[:], cdiag[:, ct, tap, :],
                            xp[:, ct, tap:tap + TA],
                            start=(tap == 0), stop=(tap == K - 1))
                    nc.scalar.activation(xh[:, ct, :], pc[:], AF.Silu,
                                         bias=cbias[:, ct:ct + 1])
                return xp, zs, xh

            def stage(widx, rhs, lane0, off, out, func, scale=1.0):
                for q in range(2):
                    pg = ps.tile([P, 2, TB], F32, tag="mm", name="pg")
                    nc.tensor.matmul(
                        pg[:], blk[:, widx, :],
                        rhs[:, lane0 + 2 * q:lane0 + 2 * q + 2, off:off + TB],
                        start=True, stop=True)
                    nc.scalar.activation(out[:, 2 * q:2 * q + 2, :], pg[:],
                                         func, bias=bias[:, widx:widx + 1],
                                         scale=scale)

            def gates_scan(j, xh, off, h, h_prev, off_prev):
                for hf in range(2):
                    p0 = 4 * hf
                    bbh = wk.tile([P, 4, TB], BF16, tag="wk", name="bbh")
                    stage(0, xh, p0, off, bbh, AF.Silu)
                    f1 = wk.tile([P, 4, TB], BF16, tag="wk", name="f1")
                    stage(1, bbh, 0, 0, f1, AF.Tanh)
                    f2 = wk.tile([P, 4, TB], BF16, tag="wk", name="f2")
                    stage(2, bbh, 0, 0, f2, AF.Tanh)
                    tt = wk.tile([P, 4, TB], BF16, tag="wk", name="tt")
                    stage(3, bbh, 0, 0, tt, AF.Tanh, scale=0.5)
                    td = wk.tile([P, 4, TB], BF16, tag="wk", name="td")
                    stage(4, bbh, 0, 0, td, AF.Tanh, scale=0.5)

                    vv = wk.tile([P, 4, TB], BF16, tag="wk", name="vv")
                    nc.vector.tensor_tensor(vv[:], f2[:], f1[:], ALU.subtract)
                    ss = wk.tile([P, 4, TB], BF16, tag="wk", name="ss")
                    nc.gpsimd.tensor_tensor(ss[:], f1[:], f2[:], ALU.add)
                    ww = wk.tile([P, 4, TB], BF16, tag="wk", name="ww")
                    nc.vector.tensor_tensor(ww[:], tt[:], vv[:], ALU.mult)
                    cc = wk.tile([P, 4, TB], BF16, tag="wk", name="cc")
                    nc.vector.tensor_tensor(cc[:], ss[:], ww[:], ALU.add)
                    dneg = wk.tile([P, 4, TB], BF16, tag="wk", name="dneg")
                    nc.vector.tensor_scalar(dneg[:], td[:], -0.5, 0.5,
                                            ALU.mult, ALU.add)
                    dd = wk.tile([P, 4, TB], BF16, tag="wk", name="dd")
                    nc.vector.tensor_scalar(dd[:], td[:], 0.5, 0.5,
                                            ALU.mult, ALU.add)
                    ch = wk.tile([P, 4, TB], BF16, tag="wk", name="ch")
                    nc.vector.tensor_tensor(ch[:], dneg[:], cc[:], ALU.mult)
                    for lq in range(4):
                        lt = p0 + lq
                        init = (0.0 if j == 0 else
                                h_prev[:, lt, off_prev + TB - 1:off_prev + TB])
                        nc.vector.tensor_tensor_scan(
                            h[:, lt, off:off + TB], dd[:, lq, :],
                            ch[:, lq, :], init, ALU.mult, ALU.add)

            def s3_gate(j, h, zs, off):
                ghs = []
                for hf in range(2):
                    p0 = 4 * hf
                    oseq = wk.tile([P, 4, TB], BF16, tag="gh", bufs=5,
                                   name="oseq")
                    for q in range(2):
                        pg = ps.tile([P, 2, TB], F32, tag="mm", name="pg")
                        nc.tensor.matmul(
                            pg[:], blk[:, 5, :],
                            h[:, p0 + 2 * q:p0 + 2 * q + 2, off:off + TB],
                            start=True, stop=True)
                        nc.scalar.activation(oseq[:, 2 * q:2 * q + 2, :],
                                             pg[:], AF.Identity,
                                             bias=bias[:, 5:6])
                    gh = wk.tile([P, 4, TB], BF16, tag="gh", bufs=5, name="gh")
                    nc.gpsimd.tensor_tensor(
                        gh[:], oseq[:], zs[:, p0:p0 + 4, off:off + TB],
                        ALU.mult)
                    ghs.append(gh)
                return ghs

            def outproj(j, ghs):
                for st in range(TB // P):
                    py = psy.tile([P, H], F32, tag="py", name="py")
                    for kt in range(8):
                        lh = ghs[kt // 4][:, kt % 4, st * P:(st + 1) * P]
                        nc.tensor.matmul(py[:, 0:512], lh,
                                         w_out[:, kt, 0:512],
                                         start=(kt == 0), stop=(kt == 7))
                        nc.tensor.matmul(py[:, 512:H], lh,
                                         w_out[:, kt, 512:H],
                                         start=(kt == 0), stop=(kt == 7))
                    ysb = sb.tile([P, H], BF16, tag="ysb", bufs=4, name="ysb")
                    nc.vector.tensor_copy(ysb[:], py[:])
                    nc.scalar.dma_start(
                        y_d[j * TB + st * P:j * TB + (st + 1) * P, :], ysb[:])

            # ---- software-pipelined main loop ----
            # phase1 runs one full chunk ahead of the gate/scan sub-chunks.
            xT_cur = dma_xT(0)
            xT_next = dma_xT(1)
            xp_prev, zs_c, xh_c = phase1(0, xT_cur, None)
            chunk = {0: (zs_c, xh_c)}
            h_prev_info = (None, 0)
            pend_s3 = {}   # j -> (h, zs, off)
            pend_op = {}   # j -> ghs
            for i in range(NA):
                if i + 2 < NA:
                    xT_after = dma_xT(i + 2)
                else:
                    xT_after = None
                if i + 1 < NA:
                    xp_prev, zs_n, xh_n = phase1(i + 1, xT_next, xp_prev)
                    chunk[i + 1] = (zs_n, xh_n)
                xT_next = xT_after
                zs_i, xh_i = chunk[i]
                h = sb.tile([P, 8, TA], BF16, tag="h", bufs=2, name="h")
                for s_ in range(2):
                    j = 2 * i + s_
                    off = s_ * TB
                    hp, offp = h_prev_info
                    gates_scan(j, xh_i, off, h, hp, offp)
                    h_prev_info = (h, off)
                    pend_s3[j] = (h, zs_i, off)
                    # out_proj before s3: gives the j-1 scan chain (ACT->DVE)
                    # a full out_proj of slack before PE needs h(j-1)
                    if j - 2 in pend_op:
                        outproj(j - 2, pend_op.pop(j - 2))
                    if j - 1 in pend_s3:
                        jj = j - 1
                        hh, zz, oo = pend_s3.pop(jj)
                        pend_op[jj] = s3_gate(jj, hh, zz, oo)
                chunk.pop(i - 1, None)
            # drain
            jj = NB - 1
            hh, zz, oo = pend_s3.pop(jj)
            pend_op[jj] = s3_gate(jj, hh, zz, oo)
            outproj(NB - 2, pend_op.pop(NB - 2))
            outproj(NB - 1, pend_op.pop(NB - 1))

    nc.compile()
    return nc


def _to_bf16(a):
    import ml_dtypes
    return np.asarray(a, np.float32).astype(ml_dtypes.bfloat16)


def _prep_shared(inputs):
    """Host-side preprocessing of the shared (weight) tensors."""
    f32 = np.float32
    in_proj_w = np.asarray(inputs["in_proj_w"], f32)
    conv_w = np.asarray(inputs["conv_w"], f32)
    conv_b = np.asarray(inputs["conv_b"], f32)

    w_in = in_proj_w.reshape(8, P, 2 * H).transpose(1, 0, 2)
    w_out = np.asarray(inputs["out_proj_w"], f32).reshape(8, P, H).transpose(1, 0, 2)

    cdiag = np.zeros((8, K, P, P), f32)
    rng = np.arange(P)
    for ct in range(8):
        for tap in range(K):
            cdiag[ct, tap, rng, rng] = conv_w[ct * P:(ct + 1) * P, 0, tap]
    cdiag = cdiag.transpose(2, 0, 1, 3)  # (P, 8, K, P)
    cbias = conv_b.reshape(8, P).T  # (P, 8)

    def blk2(w):
        o = np.zeros((P, P), f32)
        o[:64, :64] = w
        o[64:, 64:] = w
        return o

    blk = np.stack([
        blk2(np.asarray(inputs["bb_w"], f32)),
        blk2(np.asarray(inputs["f1_w"], f32)),
        blk2(np.asarray(inputs["f2_w"], f32)),
        blk2(np.asarray(inputs["tau_a_w"], f32)),
        blk2(np.asarray(inputs["decay_w"], f32)),
        blk2(np.asarray(inputs["state_out_w"], f32) * 0.5),
    ], axis=1)  # (P, 6, P)

    def t2(v):
        return np.tile(np.asarray(v, f32), 2)

    bias = np.stack([
        t2(inputs["bb_b"]),
        t2(inputs["f1_b"]),
        t2(inputs["f2_b"]),
        0.5 * (t2(inputs["tau_a_b"]) + t2(inputs["tau_b"])),
        0.5 * t2(inputs["decay_b"]),
        t2(inputs["state_out_b"]),
    ], axis=1)  # (P, 6)

    return {
        "w_in": np.ascontiguousarray(_to_bf16(w_in)),
        "w_out": np.ascontiguousarray(_to_bf16(w_out)),
        "cdiag": np.ascontiguousarray(_to_bf16(cdiag)),
        "cbias": np.ascontiguousarray(cbias),
        "blk": np.ascontiguousarray(_to_bf16(blk)),
        "bias": np.ascontiguousarray(bias),
    }


def _make_in_maps(inputs):
    shared = _prep_shared(inputs)
    x = _to_bf16(inputs["x"])
    in_maps = []
    for b in range(N_CORES):
        m = dict(shared)
        m["x"] = np.ascontiguousarray(x[b])
        in_maps.append(m)
    return in_maps


def kernel(**inputs) -> np.ndarray:
    from concourse import bass_utils

    if "nc" not in _CACHE:
        _CACHE["nc"] = _build_program()
    nc = _CACHE["nc"]

    in_maps = _make_in_maps(inputs)
    res = bass_utils.run_bass_kernel_spmd(nc, in_maps, core_ids=list(range(N_CORES)))
    out = np.stack([np.asarray(res.results[b]["y"]).astype(np.float32)
                    for b in range(N_CORES)], axis=0)
    return out


# tau/decay note: sigmoid(u) = 0.5 + 0.5*tanh(u/2). ACT computes
# tanh(0.5*psum + 0.5*b) via scale=0.5 and a pre-halved bias column. The
# remaining affines use only fast-mode DVE ops:
#   2*candidate = C = (f1+f2) + T*(f2-f1)
#   scan input ch = (1-d)*C, so the scan state is g = 2*h and 0.5 is
#   folded into the state_out weights host-side.
